# revision 8
# baseline (speedup 1.0000x reference)
"""Int8-dequant linear kernel for Trainium2 (8 NeuronCores, tensor-parallel).

Computes  y = x @ (qweight * weight_scale)^T + bias
  x:       [4096, 4096]  f32
  qweight: [16384, 4096] int8 (or int32)
  bias:    [16384]       f32
  y:       [4096, 16384] f32

Sharding: column-parallel over out_features — each of the 8 cores owns a
[2048, 4096] slice of qweight and the matching bias slice; x is replicated.
Each core computes its [4096, 2048] output slice; the host concatenates.

Math/layout choices:
  * weight_scale is folded into x on the host: y = (x*s) @ qw^T + bias.
  * Both matmul operands are cast to bf16. int8 weight values are exactly
    representable in bf16; x*s loses ~2^-9 relative — accumulation is fp32
    in PSUM, so the end-to-end relative error is ~1e-3.
  * Operands are pre-arranged host-side into partition-major layouts so the
    contraction dim (d = ko*128 + ki) lands on SBUF partitions (ki) and every
    DMA is >=4KB contiguous per partition.

Device kernel (per core): the whole weight shard lives in SBUF as bf16
([128, 32, 2048] = 128KB/partition). For each of 32 token tiles, stream the
x^T tile [128, 32, 128], run 32 (k) x 4 (n) matmuls of N=512 accumulating
into 4 PSUM banks, then evict with a fused psum+bias tensor_add and DMA out.
"""

import numpy as np
import ml_dtypes

import concourse.bass as bass
import concourse.mybir as mybir
import concourse.tile as tile
from concourse import bacc
from concourse.bass_utils import run_bass_kernel_spmd

N_CORES = 8
TOKENS, D_IN, D_OUT = 4096, 4096, 16384
O_SH = D_OUT // N_CORES  # 2048 out-features per core
P = 128
KO = D_IN // P  # 32 contraction chunks
MO = TOKENS // P  # 32 token tiles
N_FREE = 512  # matmul moving free dim == one PSUM bank of f32
N_TILES = O_SH // N_FREE  # 4

_cache: dict = {}


def _build_bass():
    bf16 = mybir.dt.bfloat16
    f32 = mybir.dt.float32
    # Bacc (not raw Bass): its compile() legalizes multi-wait instructions
    # via EventSemaphore hoisting — engine instructions only carry one
    # embedded sync-wait on trn2.
    nc = bacc.Bacc(
        "TRN2", target_bir_lowering=False, debug=False, num_devices=N_CORES
    )

    # xp[ki, mo, ko, t] = (x*scale)[mo*128+t, ko*128+ki] as bf16
    x_d = nc.dram_tensor("xp", (P, MO, KO, P), bf16, kind="ExternalInput")
    # wp[ki, ko, o] = qweight_shard[o, ko*128+ki] as bf16
    w_d = nc.dram_tensor("wp", (P, KO, O_SH), bf16, kind="ExternalInput")
    # bp[0, o] = bias_shard[o] as bf16 (injected via a PSUM-preload matmul)
    b_d = nc.dram_tensor("bp", (1, O_SH), bf16, kind="ExternalInput")
    y_d = nc.dram_tensor("y", (TOKENS, O_SH), f32, kind="ExternalOutput")

    with tile.TileContext(nc) as tc:
        with (
            tc.tile_pool(name="wpool", bufs=KO) as wpool,
            tc.tile_pool(name="xpool", bufs=3) as xpool,
            tc.tile_pool(name="bpool", bufs=1) as bpool,
            tc.tile_pool(name="opool", bufs=2) as opool,
            tc.tile_pool(name="psum", bufs=8, space="PSUM") as psum_pool,
        ):
            # Bias is seeded into each PSUM bank by an initial matmul
            # (start=True):  ones[128,128]^T @ biasq  where biasq has the
            # bias on partition 0 and zeros elsewhere, i.e. out[t, o] =
            # bias[o]. The dequant matmuls then accumulate on top. This
            # keeps the eviction a 2-operand tensor_copy (a 3-operand
            # tensor_tensor can only carry one embedded sync-wait, which
            # walrus rejects when Tile needs two).
            ones_sb = bpool.tile([P, P], bf16)
            nc.any.memset(ones_sb[:], 1.0)
            biasq_sb = bpool.tile([P, O_SH], bf16)
            nc.any.memset(biasq_sb[:], 0.0)
            nc.sync.dma_start(biasq_sb[0:1, :], b_d[:])

            # Whole weight shard resident in SBUF, one tile per k-chunk so
            # matmuls of chunk ko only depend on that chunk's DMA.
            w_tiles = []
            for ko in range(KO):
                wt = wpool.tile([P, O_SH], bf16, tag="w")
                nc.sync.dma_start(wt[:], w_d[:, ko])
                w_tiles.append(wt)

            for m in range(MO):
                x_sb = xpool.tile([P, KO, P], bf16, tag="x")
                nc.sync.dma_start(x_sb[:], x_d[:, m])
                psums = [
                    psum_pool.tile([P, N_FREE], f32, tag="ps", name=f"ps{n}")
                    for n in range(N_TILES)
                ]
                for n in range(N_TILES):
                    nc.tensor.matmul(
                        psums[n][:],
                        ones_sb[:],
                        biasq_sb[:, n * N_FREE : (n + 1) * N_FREE],
                        start=True,
                        stop=False,
                    )
                for ko in range(KO):
                    lhsT = x_sb[:, ko]
                    for n in range(N_TILES):
                        nc.tensor.matmul(
                            psums[n][:],
                            lhsT,
                            w_tiles[ko][:, n * N_FREE : (n + 1) * N_FREE],
                            start=False,
                            stop=(ko == KO - 1),
                        )
                out_sb = opool.tile([P, O_SH], f32, tag="o")
                for n in range(N_TILES):
                    sl = slice(n * N_FREE, (n + 1) * N_FREE)
                    nc.any.tensor_copy(out=out_sb[:, sl], in_=psums[n][:])
                nc.sync.dma_start(y_d[m * P : (m + 1) * P, :], out_sb[:])

    nc.compile()
    return nc


def _prep_in_maps(x, qweight, weight_scale, bias):
    bf16 = ml_dtypes.bfloat16
    scale = np.float32(np.asarray(weight_scale))
    xs = (np.asarray(x, dtype=np.float32) * scale).astype(bf16)  # [T, D]
    # [T=(mo t), D=(ko ki)] -> [ki, mo, ko, t]
    x_prep = np.ascontiguousarray(xs.reshape(MO, P, KO, P).transpose(3, 0, 2, 1))

    qw = np.asarray(qweight)
    b = np.asarray(bias, dtype=np.float32)
    in_maps = []
    for c in range(N_CORES):
        qc = qw[c * O_SH : (c + 1) * O_SH, :].astype(bf16)  # [O_SH, D], exact
        # [D=(ko ki), O] -> [ki, ko, o]
        w_prep = np.ascontiguousarray(qc.T.reshape(KO, P, O_SH).transpose(1, 0, 2))
        b_prep = b[c * O_SH : (c + 1) * O_SH].astype(bf16).reshape(1, O_SH)
        in_maps.append({"xp": x_prep, "wp": w_prep, "bp": b_prep})
    return in_maps


def _run(inputs, **kwargs):
    if "nc" not in _cache:
        _cache["nc"] = _build_bass()
    nc = _cache["nc"]
    in_maps = _prep_in_maps(**inputs)
    res = run_bass_kernel_spmd(nc, in_maps, core_ids=list(range(N_CORES)), **kwargs)
    y = np.concatenate([res.results[c]["y"] for c in range(N_CORES)], axis=1)
    return y, res


def kernel(**inputs) -> np.ndarray:
    y, _ = _run(inputs)
    return y


# revision 10
# speedup vs baseline: 1.0387x; 1.0387x over previous
"""Int8-dequant linear kernel for Trainium2 (8 NeuronCores, tensor-parallel).

Computes  y = x @ (qweight * weight_scale)^T + bias
  x:       [4096, 4096]  f32
  qweight: [16384, 4096] int8 (or int32)
  bias:    [16384]       f32
  y:       [4096, 16384] f32

Sharding: column-parallel over out_features — each of the 8 cores owns a
[2048, 4096] slice of qweight and the matching bias slice; x is replicated.
Each core computes its [4096, 2048] output slice; the host concatenates.

Math/layout choices:
  * weight_scale is folded into x on the host: y = (x*s) @ qw^T + bias.
  * Both matmul operands are cast to bf16. int8 weight values are exactly
    representable in bf16; x*s loses ~2^-9 relative — accumulation is fp32
    in PSUM, so the end-to-end relative error is ~1e-3.
  * Operands are pre-arranged host-side into partition-major layouts so the
    contraction dim (d = ko*128 + ki) lands on SBUF partitions (ki) and every
    DMA is >=4KB contiguous per partition.

Device kernel (per core): the whole weight shard lives in SBUF as bf16
([128, 32, 2048] = 128KB/partition). For each of 32 token tiles, stream the
x^T tile [128, 32, 128], run 32 (k) x 4 (n) matmuls of N=512 accumulating
into 4 PSUM banks, then evict with a fused psum+bias tensor_add and DMA out.
"""

import numpy as np
import ml_dtypes

import concourse.bass as bass
import concourse.mybir as mybir
import concourse.tile as tile
from concourse import bacc
from concourse.bass_utils import run_bass_kernel_spmd

N_CORES = 8
TOKENS, D_IN, D_OUT = 4096, 4096, 16384
O_SH = D_OUT // N_CORES  # 2048 out-features per core
P = 128
KO = D_IN // P  # 32 contraction chunks
MO = TOKENS // P  # 32 token tiles
N_FREE = 512  # matmul moving free dim == one PSUM bank of f32
N_TILES = O_SH // N_FREE  # 4

_cache: dict = {}


def _build_bass():
    bf16 = mybir.dt.bfloat16
    f32 = mybir.dt.float32
    # Bacc (not raw Bass): its compile() legalizes multi-wait instructions
    # via EventSemaphore hoisting — engine instructions only carry one
    # embedded sync-wait on trn2.
    nc = bacc.Bacc(
        "TRN2", target_bir_lowering=False, debug=False, num_devices=N_CORES
    )

    # xp[ki, mo, ko, t] = (x*scale)[mo*128+t, ko*128+ki] as bf16
    x_d = nc.dram_tensor("xp", (P, MO, KO, P), bf16, kind="ExternalInput")
    # wp[ki, ko, o] = qweight_shard[o, ko*128+ki] as bf16
    w_d = nc.dram_tensor("wp", (P, KO, O_SH), bf16, kind="ExternalInput")
    # bp[p, o] = bias_shard[o] replicated across partitions
    b_d = nc.dram_tensor("bp", (P, O_SH), f32, kind="ExternalInput")
    y_d = nc.dram_tensor("y", (TOKENS, O_SH), f32, kind="ExternalOutput")

    with tile.TileContext(nc) as tc:
        with (
            tc.tile_pool(name="wpool", bufs=KO) as wpool,
            tc.tile_pool(name="xpool", bufs=3) as xpool,
            tc.tile_pool(name="bpool", bufs=1) as bpool,
            tc.tile_pool(name="opool", bufs=2) as opool,
            tc.tile_pool(name="psum", bufs=8, space="PSUM") as psum_pool,
        ):

            def load_x(m):
                t = xpool.tile([P, KO, P], bf16, tag="x", name=f"x_m{m}")
                nc.sync.dma_start(t[:], x_d[:, m])
                return t

            # Issue the first x tile's DMA before the 16MB of weight DMAs
            # so the first matmul group isn't queued behind them.
            x_first = load_x(0)

            # Whole weight shard resident in SBUF, one tile per k-chunk so
            # matmuls of chunk ko only depend on that chunk's DMA.
            w_tiles = []
            for ko in range(KO):
                wt = wpool.tile([P, O_SH], bf16, tag="w")
                nc.sync.dma_start(wt[:], w_d[:, ko])
                w_tiles.append(wt)

            bias_sb = bpool.tile([P, O_SH], f32)
            nc.sync.dma_start(bias_sb[:], b_d[:])

            for m in range(MO):
                x_sb = x_first if m == 0 else load_x(m)
                psums = [
                    psum_pool.tile([P, N_FREE], f32, tag="ps", name=f"ps{n}")
                    for n in range(N_TILES)
                ]
                for ko in range(KO):
                    lhsT = x_sb[:, ko]
                    for n in range(N_TILES):
                        nc.tensor.matmul(
                            psums[n][:],
                            lhsT,
                            w_tiles[ko][:, n * N_FREE : (n + 1) * N_FREE],
                            start=(ko == 0),
                            stop=(ko == KO - 1),
                        )
                # Fused eviction: out = psum + bias (bias replicated across
                # partitions). Bacc's event-semaphore legalization handles
                # the multi-wait 3-operand tensor_tensor.
                out_sb = opool.tile([P, O_SH], f32, tag="o")
                for n in range(N_TILES):
                    sl = slice(n * N_FREE, (n + 1) * N_FREE)
                    nc.vector.tensor_add(out_sb[:, sl], psums[n][:], bias_sb[:, sl])
                nc.sync.dma_start(y_d[m * P : (m + 1) * P, :], out_sb[:])

    nc.compile()
    return nc


def _prep_in_maps(x, qweight, weight_scale, bias):
    bf16 = ml_dtypes.bfloat16
    scale = np.float32(np.asarray(weight_scale))
    xs = (np.asarray(x, dtype=np.float32) * scale).astype(bf16)  # [T, D]
    # [T=(mo t), D=(ko ki)] -> [ki, mo, ko, t]
    x_prep = np.ascontiguousarray(xs.reshape(MO, P, KO, P).transpose(3, 0, 2, 1))

    qw = np.asarray(qweight)
    b = np.asarray(bias, dtype=np.float32)
    in_maps = []
    for c in range(N_CORES):
        qc = qw[c * O_SH : (c + 1) * O_SH, :].astype(bf16)  # [O_SH, D], exact
        # [D=(ko ki), O] -> [ki, ko, o]
        w_prep = np.ascontiguousarray(qc.T.reshape(KO, P, O_SH).transpose(1, 0, 2))
        b_prep = np.ascontiguousarray(
            np.broadcast_to(b[c * O_SH : (c + 1) * O_SH], (P, O_SH))
        )
        in_maps.append({"xp": x_prep, "wp": w_prep, "bp": b_prep})
    return in_maps


def _run(inputs, **kwargs):
    if "nc" not in _cache:
        _cache["nc"] = _build_bass()
    nc = _cache["nc"]
    in_maps = _prep_in_maps(**inputs)
    res = run_bass_kernel_spmd(nc, in_maps, core_ids=list(range(N_CORES)), **kwargs)
    y = np.concatenate([res.results[c]["y"] for c in range(N_CORES)], axis=1)
    return y, res


def kernel(**inputs) -> np.ndarray:
    y, _ = _run(inputs)
    return y


# revision 12
# speedup vs baseline: 1.0686x; 1.0288x over previous
"""Int8-dequant linear kernel for Trainium2 (8 NeuronCores, tensor-parallel).

Computes  y = x @ (qweight * weight_scale)^T + bias
  x:       [4096, 4096]  f32
  qweight: [16384, 4096] int8 (or int32)
  bias:    [16384]       f32
  y:       [4096, 16384] f32

Sharding: column-parallel over out_features — each of the 8 cores owns a
[2048, 4096] slice of qweight and the matching bias slice; x is replicated.
Each core computes its [4096, 2048] output slice; the host concatenates.

Math/layout choices:
  * weight_scale is folded into x on the host: y = (x*s) @ qw^T + bias.
  * Both matmul operands are cast to bf16. int8 weight values are exactly
    representable in bf16; x*s loses ~2^-9 relative — accumulation is fp32
    in PSUM, so the end-to-end relative error is ~1e-3.
  * Operands are pre-arranged host-side into partition-major layouts so the
    contraction dim (d = ko*128 + ki) lands on SBUF partitions (ki) and every
    DMA is >=4KB contiguous per partition.

Device kernel (per core): the whole weight shard lives in SBUF as bf16
([128, 32, 2048] = 128KB/partition). For each of 32 token tiles, stream the
x^T tile [128, 32, 128], run 32 (k) x 4 (n) matmuls of N=512 accumulating
into 4 PSUM banks, then evict with a fused psum+bias tensor_add and DMA out.
"""

import numpy as np
import ml_dtypes

import concourse.bass as bass
import concourse.mybir as mybir
import concourse.tile as tile
from concourse import bacc
from concourse.bass_utils import run_bass_kernel_spmd

N_CORES = 8
TOKENS, D_IN, D_OUT = 4096, 4096, 16384
O_SH = D_OUT // N_CORES  # 2048 out-features per core
P = 128
KO = D_IN // P  # 32 contraction chunks
MO = TOKENS // P  # 32 token tiles
N_FREE = 512  # matmul moving free dim == one PSUM bank of f32
N_TILES = O_SH // N_FREE  # 4

_cache: dict = {}


def _build_bass():
    bf16 = mybir.dt.bfloat16
    f32 = mybir.dt.float32
    # Bacc (not raw Bass): its compile() legalizes multi-wait instructions
    # via EventSemaphore hoisting — engine instructions only carry one
    # embedded sync-wait on trn2.
    nc = bacc.Bacc(
        "TRN2", target_bir_lowering=False, debug=False, num_devices=N_CORES
    )

    # xp[ki, mo, ko, t] = (x*scale)[mo*128+t, ko*128+ki] as bf16
    x_d = nc.dram_tensor("xp", (P, MO, KO, P), bf16, kind="ExternalInput")
    # wp[ki, ko, o] = qweight_shard[o, ko*128+ki] as bf16
    w_d = nc.dram_tensor("wp", (P, KO, O_SH), bf16, kind="ExternalInput")
    # bp[p, o] = bias_shard[o] replicated across partitions
    b_d = nc.dram_tensor("bp", (P, O_SH), f32, kind="ExternalInput")
    y_d = nc.dram_tensor("y", (TOKENS, O_SH), f32, kind="ExternalOutput")

    with tile.TileContext(nc) as tc:
        with (
            tc.tile_pool(name="wpool", bufs=KO) as wpool,
            tc.tile_pool(name="xpool", bufs=4) as xpool,
            tc.tile_pool(name="bpool", bufs=1) as bpool,
            tc.tile_pool(name="opool", bufs=8) as opool,
            tc.tile_pool(name="psum", bufs=8, space="PSUM") as psum_pool,
        ):

            def load_x(m):
                t = xpool.tile([P, KO, P], bf16, tag="x", name=f"x_m{m}")
                nc.sync.dma_start(t[:], x_d[:, m])
                return t

            def alloc_psums(label):
                return [
                    psum_pool.tile([P, N_FREE], f32, tag="ps", name=f"ps_{label}_{n}")
                    for n in range(N_TILES)
                ]

            def mm_group(psums, x_sb, ko, start, stop):
                lhsT = x_sb[:, ko]
                for n in range(N_TILES):
                    nc.tensor.matmul(
                        psums[n][:],
                        lhsT,
                        w_tiles[ko][:, n * N_FREE : (n + 1) * N_FREE],
                        start=start,
                        stop=stop,
                    )

            def evict(psums, m):
                # Fused eviction out = psum + bias (bias replicated across
                # partitions); per-n stores so DMA overlaps later evictions.
                # Bacc's event-semaphore legalization handles the multi-wait
                # 3-operand tensor_tensor.
                for n in range(N_TILES):
                    sl = slice(n * N_FREE, (n + 1) * N_FREE)
                    o_sb = opool.tile([P, N_FREE], f32, tag="o", name=f"o_{m}_{n}")
                    nc.vector.tensor_add(o_sb[:], psums[n][:], bias_sb[:, sl])
                    nc.sync.dma_start(y_d[m * P : (m + 1) * P, sl], o_sb[:])

            # DMA order: the tensors the first matmuls/evictions need go
            # first (x0, x1, bias), then the 16MB weight load. HBM-per-core
            # is ~358GB/s, so the weight load takes ~45us; the first two
            # token tiles are fused into one k-loop below so PE consumes
            # weight chunks slower (~1.7us/chunk) than DMA delivers them
            # (~1.4us/chunk) and never stalls on the weight stream.
            x_tiles = {0: load_x(0), 1: load_x(1)}
            bias_sb = bpool.tile([P, O_SH], f32)
            nc.sync.dma_start(bias_sb[:], b_d[:])

            # Whole weight shard resident in SBUF, one tile per k-chunk so
            # matmuls of chunk ko only depend on that chunk's DMA.
            w_tiles = []
            for ko in range(KO):
                wt = wpool.tile([P, O_SH], bf16, tag="w")
                nc.sync.dma_start(wt[:], w_d[:, ko])
                w_tiles.append(wt)

            # Fused pair phase: m=0 and m=1 share one k-loop (8 PSUM banks).
            ps0, ps1 = alloc_psums("a"), alloc_psums("b")
            for ko in range(KO):
                mm_group(ps0, x_tiles[0], ko, start=(ko == 0), stop=(ko == KO - 1))
                mm_group(ps1, x_tiles[1], ko, start=(ko == 0), stop=(ko == KO - 1))
            evict(ps0, 0)
            evict(ps1, 1)

            # Steady state: one m per k-loop, 4+4 PSUM double buffering.
            for m in range(2, MO):
                x_sb = load_x(m)
                psums = alloc_psums("s")
                for ko in range(KO):
                    mm_group(psums, x_sb, ko, start=(ko == 0), stop=(ko == KO - 1))
                evict(psums, m)

    nc.compile()
    return nc


def _prep_in_maps(x, qweight, weight_scale, bias):
    bf16 = ml_dtypes.bfloat16
    scale = np.float32(np.asarray(weight_scale))
    xs = (np.asarray(x, dtype=np.float32) * scale).astype(bf16)  # [T, D]
    # [T=(mo t), D=(ko ki)] -> [ki, mo, ko, t]
    x_prep = np.ascontiguousarray(xs.reshape(MO, P, KO, P).transpose(3, 0, 2, 1))

    qw = np.asarray(qweight)
    b = np.asarray(bias, dtype=np.float32)
    in_maps = []
    for c in range(N_CORES):
        qc = qw[c * O_SH : (c + 1) * O_SH, :].astype(bf16)  # [O_SH, D], exact
        # [D=(ko ki), O] -> [ki, ko, o]
        w_prep = np.ascontiguousarray(qc.T.reshape(KO, P, O_SH).transpose(1, 0, 2))
        b_prep = np.ascontiguousarray(
            np.broadcast_to(b[c * O_SH : (c + 1) * O_SH], (P, O_SH))
        )
        in_maps.append({"xp": x_prep, "wp": w_prep, "bp": b_prep})
    return in_maps


def _run(inputs, **kwargs):
    if "nc" not in _cache:
        _cache["nc"] = _build_bass()
    nc = _cache["nc"]
    in_maps = _prep_in_maps(**inputs)
    res = run_bass_kernel_spmd(nc, in_maps, core_ids=list(range(N_CORES)), **kwargs)
    y = np.concatenate([res.results[c]["y"] for c in range(N_CORES)], axis=1)
    return y, res


def kernel(**inputs) -> np.ndarray:
    y, _ = _run(inputs)
    return y
